# revision 9
# baseline (speedup 1.0000x reference)
"""RWKV6 block (nn_Block6_90572270338515) on 8 TRN2 NeuronCores.

Strategy: 8-way token sharding — core (b, tq) owns tokens [tq*256, (tq+1)*256)
of batch b. All weights are streamed to every core (host-pretransposed,
pre-tiled, bf16). Activations live C-major ([128 part, 16 cchunk, cols]) so
every matmul contracts over the partition dim with zero on-device transposes
of activations. The WKV6 scan is chunked (L=64) into decay-weighted matmuls;
cross-core state is exchanged with one small AllGather (replica groups
{0..3}, {4..7}) of per-core chunk summaries (D, dS).

Per-core slot layout: slot i = global token s-2+i (s = tq*256). Slot 0 only
feeds the token-shift of slot 1; WKV runs over slots [1, 258) in chunks of 64
starting at col 1, with a 5th chunk covering the final owned token (+ zero
padding to col 321). For tq==0, slots 0/1 are dummies: k/w are masked to zero
at col 1 (kwmask) and the token-shift at slot 2 is blended to time_mix_shift
(blend0).

Self-contained: needs only concourse (on sys.path in the runtime image),
numpy, ml_dtypes.
"""
import numpy as np
import ml_dtypes

import concourse.bass as bass
import concourse.mybir as mybir
import concourse.tile as tile
from concourse import bacc
from concourse.bass_utils import run_bass_kernel_spmd

F32 = mybir.dt.float32
BF16 = mybir.dt.bfloat16
AF = mybir.ActivationFunctionType
OP = mybir.AluOpType
BC = bass.broadcast_tensor_aps

B, T, C, H, K = 2, 1024, 2048, 32, 64
D_MIX, D_DEC, FF = 32, 64, 7168
GN_EPS = 1e-5 * 8 ** 2
NCORE = 8
NCC = C // 128          # 16 C-chunks
NFC = FF // 128         # 56 FF-chunks
TQ = 4                  # T-chunks per batch
TOK = T // TQ           # owned tokens per core (256)
NT = TOK + 2            # slots: tokens [s-2, e) -> 258
NTF = 321               # padded buffer: WKV chunk l at cols [1+64l, 65+64l)
L = 64                  # WKV sub-chunk
NL = 5                  # sub-chunks (4 published + 1 for last owned token)
AGW = 65                # per-cc words published per core: 64 dS cols + 1 D
NO = TOK                # owned token columns in phase D
NC_ = NT - 1            # 257: token columns slots [1, 258)

bfc = lambda a: np.asarray(a, dtype=ml_dtypes.bfloat16)


# ---------------------------------------------------------------------------
# Device program
# ---------------------------------------------------------------------------

def build(debug: bool = False):
    nc = bacc.Bacc(num_devices=NCORE)

    dp = lambda n, sh, dt: nc.declare_dram_parameter(n, list(sh), dt, isOutput=False)
    do = lambda n, sh, dt: nc.declare_dram_parameter(n, list(sh), dt, isOutput=True)

    io = {}
    io["xT"] = dp("xT", [C, NT], F32)
    io["s0"] = dp("s0", [C, K], F32)
    io["tms"] = dp("tms", [C, 1], F32)
    io["cms"] = dp("cms", [C, 1], F32)
    io["kwmask"] = dp("kwmask", [128, NT], F32)
    io["kwmaskn"] = dp("kwmaskn", [128, NT], F32)
    io["blend0"] = dp("blend0", [128, 1], F32)
    io["pmask"] = dp("pmask", [128, 3], F32)

    for n in ("wr", "wk", "wv", "wg", "wo", "fwr"):
        io[n] = dp(n, [NCC, NCC * 128, 128], BF16)
    io["fwk"] = dp("fwk", [NFC, NCC * 128, 128], BF16)
    io["fwv"] = dp("fwv", [NCC, NFC * 128, 128], BF16)
    io["maa_w1"] = dp("maa_w1", [C, 5 * D_MIX], BF16)
    io["maa_w2a"] = dp("maa_w2a", [5, D_MIX + 1, C], BF16)
    io["td_w1"] = dp("td_w1", [C, D_DEC], BF16)
    io["td_w2a"] = dp("td_w2a", [D_DEC + 1, C], BF16)

    for n in ("maa_x", "u_col", "lnx_w", "lnx_b", "ln1_w", "ln1_b",
              "ln2_w", "ln2_b", "fmaa_k", "fmaa_r"):
        io[n] = dp(n, [C, 1], F32)

    io["ones128"] = dp("ones128", [128, 1], F32)
    io["onesrow"] = dp("onesrow", [1, 128], F32)
    io["ind_m"] = dp("ind_m", [NCC, 128, 32], F32)
    io["ind_mT"] = dp("ind_mT", [NCC, 32, 128], F32)
    io["um2"] = dp("um2", [128, L], F32)
    io["i2"] = dp("i2", [128, L], F32)
    io["id128"] = dp("id128", [128, 128], BF16)

    io["outx"] = do("outx", [C, TOK], F32)
    io["nshift"] = do("nshift", [C, 1], F32)
    io["ncm"] = do("ncm", [C, 1], F32)
    io["nwkv"] = do("nwkv", [C, K], F32)

    io["dbg"] = {}
    if debug:
        for n, sh in [("d_xn", [C, NT]), ("d_k", [C, NTF]), ("d_w", [C, NTF]),
                      ("d_o", [C, NT]), ("d_x2", [C, NC_]), ("d_sin", [C, K]),
                      ("d_yg", [C, NC_]), ("d_r", [C, NTF]), ("d_g", [C, NT]),
                      ("d_xmw", [C, NT]), ("d_xn2", [C, NC_])]:
            io["dbg"][n] = do(n, sh, F32)

    io["cc_in"] = nc.dram_tensor("cc_in", [128, NCC * AGW], F32)
    io["cc_out"] = nc.dram_tensor("cc_out", [TQ * 128, NCC * AGW], F32)

    with tile.TileContext(nc) as tc:
        _program(tc, nc, io)

    nc.compile()
    return nc


def r3(h):
    return h.ap().rearrange("(cc p) t -> p cc t", p=128)


def _program(tc, nc, io):
    P = 128
    dbg = io["dbg"]

    # ---------------- persistent consts ----------------
    pp = tc.alloc_tile_pool(name="persist", bufs=1)
    cvec = {}
    for n in ("maa_x", "u_col", "lnx_w", "lnx_b", "ln1_w", "ln1_b",
              "ln2_w", "ln2_b", "fmaa_k", "fmaa_r"):
        t = pp.tile([P, NCC, 1], F32, name=f"c_{n}")
        nc.sync.dma_start(t[:], r3(io[n]))
        cvec[n] = t
    c_ones = pp.tile([P, 1], F32, name="c_ones")
    nc.sync.dma_start(c_ones[:], io["ones128"].ap())
    c_onesrow = pp.tile([1, P], F32, name="c_onesrow")
    nc.sync.dma_start(c_onesrow[:], io["onesrow"].ap())
    c_um2 = pp.tile([P, 1, L], F32, name="c_um2")
    nc.sync.dma_start(c_um2[:, 0, :], io["um2"].ap())
    c_i2 = pp.tile([P, 1, L], F32, name="c_i2")
    nc.sync.dma_start(c_i2[:, 0, :], io["i2"].ap())
    c_id = pp.tile([P, P], BF16, name="c_id")
    nc.sync.dma_start(c_id[:], io["id128"].ap())
    c_kwm = pp.tile([P, NT], F32, name="c_kwm")
    nc.sync.dma_start(c_kwm[:], io["kwmask"].ap())
    c_kwmn = pp.tile([P, NT], F32, name="c_kwmn")
    nc.sync.dma_start(c_kwmn[:], io["kwmaskn"].ap())
    c_b0 = pp.tile([P, 1], F32, name="c_b0")
    nc.sync.dma_start(c_b0[:], io["blend0"].ap())
    c_pm = pp.tile([P, 3], F32, name="c_pm")
    nc.sync.dma_start(c_pm[:], io["pmask"].ap())
    c_tms = pp.tile([P, NCC, 1], F32, name="c_tms")
    nc.sync.dma_start(c_tms[:], r3(io["tms"]))
    c_cms = pp.tile([P, NCC, 1], F32, name="c_cms")
    nc.sync.dma_start(c_cms[:], r3(io["cms"]))
    c_indm = pp.tile([P, NCC, 32], F32, name="c_indm")
    nc.sync.dma_start(c_indm[:], io["ind_m"].ap().rearrange("cc p j -> p cc j"))
    c_indmT = pp.tile([32, NCC, P], F32, name="c_indmT")
    nc.sync.dma_start(c_indmT[:], io["ind_mT"].ap().rearrange("cc j p -> j cc p"))
    sav_shift = pp.tile([P, NCC, 1], F32, name="sav_shift")
    c_eps1 = pp.tile([P, 1], F32, name="c_eps1")
    nc.vector.memset(c_eps1[:], 1e-5)
    c_epsg = pp.tile([P, 1], F32, name="c_epsg")
    nc.vector.memset(c_epsg[:], GN_EPS)

    wpool = tc.alloc_tile_pool(name="wpool", bufs=3)
    pspool = tc.alloc_tile_pool(name="proj_ps", bufs=2, space="PSUM")
    # right-side stack: allocated in reverse-death order
    pool_gso = tc.alloc_tile_pool(name="pool_gso", bufs=1, side="right")
    pool_mrg = tc.alloc_tile_pool(name="pool_mrg", bufs=1, side="right")
    pool_lhw = tc.alloc_tile_pool(name="pool_lhw", bufs=1, side="right")
    pool_mkv = tc.alloc_tile_pool(name="pool_mkv", bufs=1, side="right")
    pool_lora = tc.alloc_tile_pool(name="pool_lora", bufs=1, side="right")
    pool_xn = tc.alloc_tile_pool(name="pool_xn", bufs=1, side="right")

    def ln_layer(po, src3, ncols, w_t, b_t, xn_out, sq_tag="ln_sq"):
        """LayerNorm over C for [P, NCC, ncols] src3 -> xn_out (fp32)."""
        sq = po.tile([P, NCC, ncols], F32, tag=sq_tag, name="ln_sq_t")
        nc.vector.tensor_tensor(out=sq[:], in0=src3[:], in1=src3[:], op=OP.mult)
        with tc.tile_pool(name="ln_ps", bufs=1, space="PSUM") as ps:
            p_sum = ps.tile([1, ncols], F32, tag="p_sum")
            p_sq = ps.tile([1, ncols], F32, tag="p_sq")
            for cc in range(NCC):
                nc.tensor.matmul(p_sum[:], c_ones[:], src3[:, cc, :],
                                 start=(cc == 0), stop=(cc == NCC - 1))
            for cc in range(NCC):
                nc.tensor.matmul(p_sq[:], c_ones[:], sq[:, cc, :],
                                 start=(cc == 0), stop=(cc == NCC - 1))
            mean = po.tile([1, ncols], F32, tag="ln_mean")
            nc.vector.tensor_scalar_mul(mean[:], p_sum[:], 1.0 / C)
            msq = po.tile([1, ncols], F32, tag="ln_msq")
            nc.vector.tensor_tensor(out=msq[:], in0=mean[:], in1=mean[:],
                                    op=OP.mult)
            var = po.tile([1, ncols], F32, tag="ln_var")
            nc.vector.scalar_tensor_tensor(
                out=var[:], in0=p_sq[:], scalar=1.0 / C, in1=msq[:],
                op0=OP.mult, op1=OP.subtract)
            std = po.tile([1, ncols], F32, tag="ln_std")
            nc.scalar.activation(std[:], var[:], AF.Sqrt, bias=c_eps1[0:1, :])
            rstd = po.tile([1, ncols], F32, tag="ln_rstd")
            nc.vector.reciprocal(rstd[:], std[:])
            p_mb = ps.tile([P, ncols], F32, tag="p_mb")
            p_rb = ps.tile([P, ncols], F32, tag="p_rb")
            nc.tensor.matmul(p_mb[:], c_onesrow[:], mean[:])
            nc.tensor.matmul(p_rb[:], c_onesrow[:], rstd[:])
            tmp = po.tile([P, ncols], F32, tag="ln_tmp")
            for cc in range(NCC):
                nc.vector.scalar_tensor_tensor(
                    out=tmp[:], in0=src3[:, cc, :], scalar=0.0,
                    in1=p_mb[:], op0=OP.add, op1=OP.subtract)
                nc.vector.scalar_tensor_tensor(
                    out=tmp[:], in0=tmp[:], scalar=w_t[:, cc, :],
                    in1=p_rb[:], op0=OP.mult, op1=OP.mult)
                nc.vector.tensor_scalar_add(
                    xn_out[:, cc, :], tmp[:], b_t[:, cc, :])

    def proj16(wparam, rhs3, ncols, evict):
        """For oc in 0..15: psum = sum_ic W[oc,ic].T @ rhs3[:, ic, :ncols]."""
        for oc in range(NCC):
            wtile = wpool.tile([P, NCC, P], BF16, tag="wtile")
            nc.sync.dma_start(
                wtile[:], wparam.ap()[oc].rearrange("(ic p) f -> p ic f", p=P))
            pm = pspool.tile([P, 512], F32, tag="proj", name="projp")[:, :ncols]
            for ic in range(NCC):
                nc.tensor.matmul(pm[:], wtile[:, ic, :], rhs3[:, ic, 0:ncols],
                                 start=(ic == 0), stop=(ic == NCC - 1))
            evict(oc, pm)

    # =================================================================
    # Phase A: LN1, token shift, LoRA mixers
    # =================================================================
    xsb = pool_xn.tile([P, NCC, NT], F32, name="xsb")
    nc.sync.dma_start(xsb[:], r3(io["xT"]))
    xn = xsb  # LN1 normalizes in place (stats are read before any write)
    ln_layer(pool_xn, xsb, NT, cvec["ln1_w"], cvec["ln1_b"], xn,
             sq_tag="xx_slot")
    nc.vector.tensor_copy(out=sav_shift[:], in_=xn[:, :, NT - 1:NT])
    if "d_xn" in dbg:
        nc.sync.dma_start(r3(dbg["d_xn"]), xn[:])

    xx = pool_xn.tile([P, NCC, NT], F32, tag="xx_slot", name="xx")
    nc.vector.tensor_tensor(out=xx[:, :, 1:NT], in0=xn[:, :, 0:NT - 1],
                            in1=xn[:, :, 1:NT], op=OP.subtract)
    nc.vector.memset(xx[:, :, 0:1], 0.0)
    # tq==0 fix at slot 2: xx[2] += blend0*((tms - xn[2]) - xx[2])
    u1 = pool_xn.tile([P, NCC, 1], F32, name="u1")
    nc.vector.tensor_tensor(out=u1[:], in0=c_tms[:], in1=xn[:, :, 2:3],
                            op=OP.subtract)
    nc.vector.tensor_tensor(out=u1[:], in0=u1[:], in1=xx[:, :, 2:3],
                            op=OP.subtract)
    u2 = pool_xn.tile([P, NCC, 1], F32, name="u2")
    for cc in range(NCC):
        nc.vector.scalar_tensor_tensor(
            out=u2[:, cc, :], in0=u1[:, cc, :], scalar=c_b0[:],
            in1=xx[:, cc, 2:3], op0=OP.mult, op1=OP.add)
    nc.vector.tensor_copy(out=xx[:, :, 2:3], in_=u2[:])

    # LoRA: hh_f = tanh((xn + xx*maa_x) @ maa_w1)[f] (+ ones row)
    bx = pool_lora.tile([P, NCC, NT], BF16, name="bx")
    for cc in range(NCC):
        nc.vector.scalar_tensor_tensor(
            out=bx[:, cc, :], in0=xx[:, cc, :], scalar=cvec["maa_x"][:, cc, :],
            in1=xn[:, cc, :], op0=OP.mult, op1=OP.add)
    w1sb = pool_lora.tile([P, NCC, 5 * D_MIX], BF16, name="w1sb")
    nc.sync.dma_start(w1sb[:],
                      io["maa_w1"].ap().rearrange("(cc p) f -> p cc f", p=P))
    hh_f = [pool_lora.tile([D_MIX + 1, NT], BF16, name=f"hh_{f}")
            for f in range(5)]
    with tc.tile_pool(name="lora_ps", bufs=1, space="PSUM") as ps:
        pA = ps.tile([P, NT], F32, tag="lora_a")
        pB = ps.tile([32, NT], F32, tag="lora_b")
        for cc in range(NCC):
            nc.tensor.matmul(pA[:], w1sb[:, cc, 0:128], bx[:, cc, :],
                             start=(cc == 0), stop=(cc == NCC - 1))
        for cc in range(NCC):
            nc.tensor.matmul(pB[:], w1sb[:, cc, 128:160], bx[:, cc, :],
                             start=(cc == 0), stop=(cc == NCC - 1))
        for f in range(4):
            nc.scalar.activation(hh_f[f][0:D_MIX, :],
                                 pA[f * D_MIX:(f + 1) * D_MIX, :], AF.Tanh)
            nc.vector.memset(hh_f[f][D_MIX:D_MIX + 1, :], 1.0)
        nc.scalar.activation(hh_f[4][0:D_MIX, :], pB[:], AF.Tanh)
        nc.vector.memset(hh_f[4][D_MIX:D_MIX + 1, :], 1.0)

    # mixers: xmix_f = xn + xx * (maa_f + hh_f @ maa_w2[f])
    w2sb = pool_lora.tile([D_MIX + 1, 5, NCC, P], BF16, name="w2sb")
    nc.sync.dma_start(w2sb[:],
                      io["maa_w2a"].ap().rearrange("f d (cc p) -> d f cc p", p=P))
    xw = pool_mkv.tile([P, NCC, NT], BF16, name="xw")
    xk = pool_mkv.tile([P, NCC, NT], BF16, name="xk")
    xv = pool_mkv.tile([P, NCC, NT], BF16, name="xv")
    xr = pool_mrg.tile([P, NCC, NT], BF16, name="xr")
    xg = pool_mrg.tile([P, NCC, NT], BF16, name="xg")
    xmix = [xw, xk, xv, xr, xg]
    with tc.tile_pool(name="mix_ps", bufs=4, space="PSUM") as ps:
        for f in range(5):
            for cc in range(NCC):
                pm = ps.tile([P, NT], F32, tag="mixmm")
                nc.tensor.matmul(pm[:], w2sb[:, f, cc, :], hh_f[f][:])
                tt = pool_lora.tile([P, NT], F32, tag="mix_tmp")
                nc.vector.tensor_tensor(out=tt[:], in0=pm[:], in1=xx[:, cc, :],
                                        op=OP.mult)
                nc.vector.tensor_tensor(out=xmix[f][:, cc, :], in0=tt[:],
                                        in1=xn[:, cc, :], op=OP.add)
    pool_xn.release()   # free xsb, xn, xx
    if "d_xmw" in dbg:
        tdmp = pool_lora.tile([P, NCC, NT], F32, tag="tdmp")
        nc.vector.tensor_copy(out=tdmp[:], in_=xw[:])
        nc.sync.dma_start(r3(dbg["d_xmw"]), tdmp[:])

    # hw = tanh(xw @ td_w1) (+ ones row) for the decay LoRA
    tdw1 = pool_lora.tile([P, NCC, D_DEC], BF16, name="tdw1")
    nc.sync.dma_start(tdw1[:],
                      io["td_w1"].ap().rearrange("(cc p) f -> p cc f", p=P))
    hw_f = pool_lhw.tile([D_DEC + 1, NT], BF16, name="hw_f")
    with tc.tile_pool(name="loraw_ps", bufs=1, space="PSUM") as ps:
        pW = ps.tile([D_DEC, NT], F32, tag="lora_w")
        for cc in range(NCC):
            nc.tensor.matmul(pW[:], tdw1[:, cc, :], xw[:, cc, :],
                             start=(cc == 0), stop=(cc == NCC - 1))
        nc.scalar.activation(hw_f[0:D_DEC, :], pW[:], AF.Tanh)
        nc.vector.memset(hw_f[D_DEC:D_DEC + 1, :], 1.0)
    pool_lora.release()

    # =================================================================
    # Phase B: projections k, v, w -> summaries -> AllGather;  r, g
    # =================================================================
    pool_kv = tc.alloc_tile_pool(name="pool_kv", bufs=1)   # kT,vT,rT,cs,Dl
    pool_w = tc.alloc_tile_pool(name="pool_w", bufs=1)     # wT (dies after scans)
    kT = pool_kv.tile([P, NCC, NTF], BF16, name="kT")
    vT = pool_kv.tile([P, NCC, NTF], BF16, name="vT")
    rT = pool_kv.tile([P, NCC, NTF], BF16, name="rT")
    wT = pool_w.tile([P, NCC, NTF], F32, name="wT")
    gsT = pool_gso.tile([P, NCC, NT], BF16, name="gsT")
    for t in (kT, vT, rT):
        nc.vector.memset(t[:, :, NT:NTF], 0.0)
    nc.vector.memset(wT[:, :, NT:NTF], 0.0)

    proj16(io["wk"], xk, NT,
           lambda oc, pm: nc.vector.tensor_tensor(
               out=kT[:, oc, 0:NT], in0=pm[:], in1=c_kwm[:], op=OP.mult))
    proj16(io["wv"], xv, NT,
           lambda oc, pm: nc.scalar.copy(vT[:, oc, 0:NT], pm[:]))
    pool_mkv.release()

    # w decay: wT = -exp(td + hw @ td_w2) * kwmask
    w2aug = pool_w.tile([D_DEC + 1, NCC, P], BF16, name="w2aug")
    nc.sync.dma_start(w2aug[:],
                      io["td_w2a"].ap().rearrange("d (cc p) -> d cc p", p=P))
    for oc in range(NCC):
        pm = pspool.tile([P, 512], F32, tag="proj", name="projp")[:, :NT]
        nc.tensor.matmul(pm[:], w2aug[:, oc, :], hw_f[:])
        wtmp = pool_w.tile([P, NT], F32, tag="wtmp")
        nc.scalar.activation(wtmp[:], pm[:], AF.Exp)
        nc.vector.tensor_tensor(out=wT[:, oc, 0:NT], in0=wtmp[:],
                                in1=c_kwmn[:], op=OP.mult)
    pool_lhw.release()

    # cumulative log-decay per chunk; Dl = per-chunk total decay
    cs = pool_kv.tile([P, NCC, NTF], F32, name="cs")
    for l in range(NL):
        sl = slice(1 + L * l, 1 + L * (l + 1))
        for cc in range(NCC):
            nc.vector.tensor_tensor_scan(
                out=cs[:, cc, sl], data0=wT[:, cc, sl], data1=wT[:, cc, sl],
                initial=0.0, op0=OP.add, op1=OP.bypass)
    if "d_w" in dbg:
        nc.sync.dma_start(r3(dbg["d_w"]), wT[:])
    Dl = pool_kv.tile([P, NCC, NL], F32, name="Dl")
    for l in range(NL):
        nc.scalar.activation(Dl[:, :, l:l + 1],
                             cs[:, :, L * (l + 1):L * (l + 1) + 1], AF.Exp)
    pool_w.release()

    # pre-collective: dS_l for all chunks (needs only k, v, w)
    pool_ds = tc.alloc_tile_pool(name="pool_ds", bufs=1)
    pool_tr = tc.alloc_tile_pool(name="pool_tr", bufs=1)
    pool_s = tc.alloc_tile_pool(name="pool_s", bufs=1)
    tp_ps = tc.alloc_tile_pool(name="tp_ps", bufs=1, space="PSUM")
    mm_ps = tc.alloc_tile_pool(name="mm_ps", bufs=2, space="PSUM")
    ds_l = [pool_ds.tile([P, NCC, L], F32, name=f"ds_{l}") for l in range(NL)]
    for l in range(NL):
        sl = slice(1 + L * l, 1 + L * (l + 1))
        lc = slice(L * (l + 1), L * (l + 1) + 1)
        expd = pool_tr.tile([P, NCC, L], F32, tag="expd")
        for cc in range(NCC):
            nc.scalar.activation(expd[:, cc, :], cs[:, cc, sl], AF.Exp,
                                 scale=-1.0, bias=cs[:, cc, lc])
        khat = pool_tr.tile([P, NCC, L], BF16, tag="khat")
        nc.vector.tensor_tensor(out=khat[:], in0=kT[:, :, sl], in1=expd[:],
                                op=OP.mult)
        p_kt = tp_ps.tile([L, NCC, P], BF16, tag="p_tp")
        for cc in range(NCC):
            nc.tensor.transpose(p_kt[:, cc, :], khat[:, cc, :], c_id[:])
        ktm = pool_tr.tile([P, NCC, P], BF16, tag="ktm")
        nc.scalar.copy(ktm[0:L, :, :], p_kt[:])
        nc.scalar.copy(ktm[L:P, :, :], p_kt[:])
        p_vt = tp_ps.tile([L, NCC, P], BF16, tag="p_tp")
        for cc in range(NCC):
            nc.tensor.transpose(p_vt[:, cc, :], vT[:, cc, sl], c_id[:])
        vtm = pool_tr.tile([P, NCC, P], BF16, tag="vtm")
        nc.scalar.copy(vtm[0:L, :, :], p_vt[:])
        nc.scalar.copy(vtm[L:P, :, :], p_vt[:])
        p_ds = mm_ps.tile([P, NCC, L], F32, tag="p_mm")
        for cc in range(NCC):
            nc.tensor.matmul(p_ds[0:L, cc, :], ktm[0:L, cc, 0:L],
                             vtm[0:L, cc, 0:L], tile_position=(0, 0))
            nc.tensor.matmul(p_ds[L:P, cc, :], ktm[L:P, cc, L:P],
                             vtm[L:P, cc, L:P], tile_position=(64, 64))
        nc.vector.tensor_copy(out=ds_l[l][:], in_=p_ds[:])

    # publish combined summary of chunks 0..3 and AllGather
    dpub = pool_ds.tile([P, NCC, 1], F32, name="dpub")
    nc.vector.tensor_tensor(out=dpub[:], in0=cs[:, :, L:L + 1],
                            in1=cs[:, :, 2 * L:2 * L + 1], op=OP.add)
    nc.vector.tensor_tensor(out=dpub[:], in0=dpub[:],
                            in1=cs[:, :, 3 * L:3 * L + 1], op=OP.add)
    nc.vector.tensor_tensor(out=dpub[:], in0=dpub[:],
                            in1=cs[:, :, 4 * L:4 * L + 1], op=OP.add)
    nc.scalar.activation(dpub[:], dpub[:], AF.Exp)
    spub = pool_ds.tile([P, NCC, L], F32, name="spub")
    nc.vector.tensor_copy(out=spub[:], in_=ds_l[0][:])
    for l in range(1, 4):
        a0, a1 = BC(spub[:], Dl[:, :, l:l + 1])
        nc.vector.tensor_tensor(out=spub[:], in0=a0, in1=a1, op=OP.mult)
        nc.vector.tensor_tensor(out=spub[:], in0=spub[:], in1=ds_l[l][:],
                                op=OP.add)
    ccv = io["cc_in"].ap().rearrange("p (cc w) -> p cc w", w=AGW)
    nc.sync.dma_start(ccv[:, :, 0:L], spub[:])
    nc.sync.dma_start(ccv[:, :, L:AGW], dpub[:])
    nc.gpsimd.collective_compute(
        "AllGather", OP.bypass,
        replica_groups=[[0, 1, 2, 3], [4, 5, 6, 7]],
        ins=[io["cc_in"].ap().opt()], outs=[io["cc_out"].ap().opt()])

    # r / g projections (independent -> overlap the collective)
    proj16(io["wr"], xr, NT,
           lambda oc, pm: nc.scalar.copy(rT[:, oc, 0:NT], pm[:]))
    proj16(io["wg"], xg, NT,
           lambda oc, pm: nc.scalar.activation(gsT[:, oc, :], pm[:], AF.Silu))
    pool_mrg.release()
    if "d_r" in dbg:
        for nm, src in (("d_r", rT), ("d_k", kT), ("d_g", None)):
            pass
        tdr = pool_tr.tile([P, NCC, NTF], F32, tag="tdr")
        nc.vector.tensor_copy(out=tdr[:], in_=rT[:])
        nc.sync.dma_start(r3(dbg["d_r"]), tdr[:])
        nc.vector.tensor_copy(out=tdr[:], in_=kT[:])
        nc.sync.dma_start(r3(dbg["d_k"]), tdr[:])
        tdg = pool_tr.tile([P, NCC, NT], F32, tag="tdg")
        nc.vector.tensor_copy(out=tdg[:], in_=gsT[:])
        nc.sync.dma_start(r3(dbg["d_g"]), tdg[:])

    # ---------------- compose S_in from gathered peers ----------------
    S0 = pool_s.tile([P, NCC, L], F32, name="S0")
    nc.sync.dma_start(S0[:], io["s0"].ap().rearrange("(cc p) v -> p cc v", p=P))
    agv = io["cc_out"].ap().rearrange("(q p) (cc w) -> q p cc w", p=P, w=AGW)
    for pr in range(3):
        peer_s = pool_s.tile([P, NCC, L], F32, tag="peer_s")
        peer_d = pool_s.tile([P, NCC, 1], F32, tag="peer_d")
        nc.sync.dma_start(peer_s[:], agv[pr, :, :, 0:L])
        nc.sync.dma_start(peer_d[:], agv[pr, :, :, L:AGW])
        deff = pool_s.tile([P, NCC, 1], F32, tag="deff")
        nc.vector.tensor_scalar_add(deff[:], peer_d[:], -1.0)
        for cc in range(NCC):
            nc.vector.tensor_scalar(
                out=deff[:, cc, :], in0=deff[:, cc, :],
                scalar1=c_pm[:, pr:pr + 1], scalar2=1.0,
                op0=OP.mult, op1=OP.add)
            nc.vector.tensor_scalar_mul(
                peer_s[:, cc, :], peer_s[:, cc, :], c_pm[:, pr:pr + 1])
        a0, a1 = BC(S0[:], deff[:])
        nc.vector.tensor_tensor(out=S0[:], in0=a0, in1=a1, op=OP.mult)
        nc.vector.tensor_tensor(out=S0[:], in0=S0[:], in1=peer_s[:], op=OP.add)
    if "d_sin" in dbg:
        nc.sync.dma_start(
            dbg["d_sin"].ap().rearrange("(cc p) v -> p cc v", p=P), S0[:])

    # ---------------- WKV output loop ----------------
    o_t = pool_gso.tile([P, NCC, NT], F32, name="o_t")
    S_cur = S0
    for l in range(NL):
        sl = slice(1 + L * l, 1 + L * (l + 1))
        ncols = L if l < NL - 1 else 1
        # decay factors for this chunk
        expnc = pool_tr.tile([P, NCC, L], F32, tag="expnc")
        nc.scalar.activation(expnc[:], cs[:, :, sl], AF.Exp, scale=-1.0)
        expe = pool_tr.tile([P, NCC, L], F32, tag="expe")
        nc.scalar.activation(expe[:, :, 1:L], cs[:, :, sl][:, :, 0:L - 1],
                             AF.Exp)
        nc.vector.memset(expe[:, :, 0:1], 1.0)
        rt_l = pool_tr.tile([P, NCC, L], BF16, tag="rt_l")
        nc.vector.tensor_tensor(out=rt_l[:], in0=rT[:, :, sl], in1=expe[:],
                                op=OP.mult)
        ktil = pool_tr.tile([P, NCC, L], BF16, tag="ktil")
        nc.vector.tensor_tensor(out=ktil[:], in0=kT[:, :, sl], in1=expnc[:],
                                op=OP.mult)
        # e for the intra diagonal: d = sum_keys (r*u*k)
        e_sl = pool_tr.tile([P, NCC, L], BF16, tag="e_sl")
        for cc in range(NCC):
            nc.vector.scalar_tensor_tensor(
                out=e_sl[:, cc, :], in0=rT[:, cc, sl],
                scalar=cvec["u_col"][:, cc, :], in1=kT[:, cc, sl],
                op0=OP.mult, op1=OP.mult)
        p_et = tp_ps.tile([L, NCC, P], BF16, tag="p_tp")
        for cc in range(NCC):
            nc.tensor.transpose(p_et[:, cc, :], e_sl[:, cc, :], c_id[:])
        dfull = pool_tr.tile([P, NCC, 1], F32, tag="dfull")
        for cc in range(NCC):
            nc.vector.reduce_sum(dfull[0:L, cc, :], p_et[:, cc, 0:L],
                                 axis=mybir.AxisListType.X)
            nc.vector.reduce_sum(dfull[L:P, cc, :], p_et[:, cc, L:P],
                                 axis=mybir.AxisListType.X)
        # v transpose (again, for the intra matmuls)
        p_vt = tp_ps.tile([L, NCC, P], BF16, tag="p_tp")
        for cc in range(NCC):
            nc.tensor.transpose(p_vt[:, cc, :], vT[:, cc, sl], c_id[:])
        vtm = pool_tr.tile([P, NCC, P], BF16, tag="vtm2")
        nc.scalar.copy(vtm[0:L, :, :], p_vt[:])
        nc.scalar.copy(vtm[L:P, :, :], p_vt[:])
        # AT[j,i] (strict upper) and diag matrix
        p_at = mm_ps.tile([P, NCC, L], F32, tag="p_mm")
        for cc in range(NCC):
            nc.tensor.matmul(p_at[0:L, cc, :], ktil[0:L, cc, :],
                             rt_l[0:L, cc, :], tile_position=(0, 0))
            nc.tensor.matmul(p_at[L:P, cc, :], ktil[L:P, cc, :],
                             rt_l[L:P, cc, :], tile_position=(64, 64))
        at_l = pool_tr.tile([P, NCC, L], BF16, tag="at_l")
        a0, a1 = BC(p_at[:], c_um2[:])
        nc.vector.tensor_tensor(out=at_l[:], in0=a0, in1=a1, op=OP.mult)
        dm_l = pool_tr.tile([P, NCC, L], BF16, tag="dm_l")
        a0, a1 = BC(c_i2[:], dfull[:])
        nc.vector.tensor_tensor(out=dm_l[:], in0=a0, in1=a1, op=OP.mult)
        # bf16 state for the inter matmul
        Sb = pool_tr.tile([P, NCC, L], BF16, tag="Sb")
        nc.vector.tensor_copy(out=Sb[:], in_=S_cur[:])
        # O^T = V_tm @ (AT + diag) + S^T-style inter, accumulated in psum
        p_ot = mm_ps.tile([P, NCC, L], F32, tag="p_mm")
        for cc in range(NCC):
            for base in (0, L):
                hs = slice(base, base + L)
                tp = (base, base)
                nc.tensor.matmul(p_ot[hs, cc, :], vtm[hs, cc, hs],
                                 at_l[hs, cc, :], start=True, stop=False,
                                 tile_position=tp)
                nc.tensor.matmul(p_ot[hs, cc, :], vtm[hs, cc, hs],
                                 dm_l[hs, cc, :], start=False, stop=False,
                                 tile_position=tp)
                nc.tensor.matmul(p_ot[hs, cc, :], Sb[hs, cc, :],
                                 rt_l[hs, cc, :], start=False, stop=True,
                                 tile_position=tp)
        nc.vector.tensor_copy(out=o_t[:, :, 1 + L * l:1 + L * l + ncols],
                              in_=p_ot[:, :, 0:ncols])
        # S_{l+1} = D_l * S_l + dS_l
        S_nxt = pool_s.tile([P, NCC, L], F32, tag="S_nxt", bufs=2)
        a0, a1 = BC(S_cur[:], Dl[:, :, l:l + 1])
        nc.vector.tensor_tensor(out=S_nxt[:], in0=a0, in1=a1, op=OP.mult)
        nc.vector.tensor_tensor(out=S_nxt[:], in0=S_nxt[:], in1=ds_l[l][:],
                                op=OP.add)
        S_cur = S_nxt
    nc.sync.dma_start(io["nwkv"].ap().rearrange("(cc p) v -> p cc v", p=P),
                      S_cur[:])
    if "d_o" in dbg:
        nc.vector.memset(o_t[:, :, 0:1], 0.0)
        nc.sync.dma_start(r3(dbg["d_o"]), o_t[:])
    pool_s.release()
    pool_tr.release()
    pool_ds.release()
    pool_kv.release()
    mm_ps.release()
    tp_ps.release()

    # =================================================================
    # Phase C: GroupNorm -> y*g -> Wo -> x2
    # =================================================================
    pc = tc.alloc_tile_pool(name="ph_c", bufs=1)
    osq = pc.tile([P, NCC, NC_], F32, name="osq")
    nc.vector.tensor_tensor(out=osq[:], in0=o_t[:, :, 1:NT],
                            in1=o_t[:, :, 1:NT], op=OP.mult)
    ygT = pc.tile([P, NCC, NC_], BF16, name="ygT")
    with tc.tile_pool(name="gn_ps", bufs=1, space="PSUM") as ps:
        p_sum = ps.tile([32, NC_], F32, tag="g_sum")
        p_sq = ps.tile([32, NC_], F32, tag="g_sq")
        for cc in range(NCC):
            nc.tensor.matmul(p_sum[:], c_indm[:, cc, :], o_t[:, cc, 1:NT],
                             start=(cc == 0), stop=(cc == NCC - 1))
        for cc in range(NCC):
            nc.tensor.matmul(p_sq[:], c_indm[:, cc, :], osq[:, cc, :],
                             start=(cc == 0), stop=(cc == NCC - 1))
        mean = pc.tile([32, NC_], F32, name="g_mean")
        nc.vector.tensor_scalar_mul(mean[:], p_sum[:], 1.0 / K)
        msq = pc.tile([32, NC_], F32, name="g_msq")
        nc.vector.tensor_tensor(out=msq[:], in0=mean[:], in1=mean[:],
                                op=OP.mult)
        var = pc.tile([32, NC_], F32, name="g_var")
        nc.vector.scalar_tensor_tensor(
            out=var[:], in0=p_sq[:], scalar=1.0 / K, in1=msq[:],
            op0=OP.mult, op1=OP.subtract)
        std = pc.tile([32, NC_], F32, name="g_std")
        nc.scalar.activation(std[:], var[:], AF.Sqrt, bias=c_epsg[0:32, :])
        rstd = pc.tile([32, NC_], F32, name="g_rstd")
        nc.vector.reciprocal(rstd[:], std[:])
        with tc.tile_pool(name="gn_bc", bufs=1, space="PSUM") as ps2:
            for cc in range(NCC):
                p_mb = ps2.tile([P, NC_], F32, tag="gb_m")
                p_rb = ps2.tile([P, NC_], F32, tag="gb_r")
                nc.tensor.matmul(p_mb[:], c_indmT[:, cc, :], mean[:])
                nc.tensor.matmul(p_rb[:], c_indmT[:, cc, :], rstd[:])
                t1 = pc.tile([P, NC_], F32, tag="gn_t1")
                nc.vector.scalar_tensor_tensor(
                    out=t1[:], in0=o_t[:, cc, 1:NT], scalar=0.0,
                    in1=p_mb[:], op0=OP.add, op1=OP.subtract)
                nc.vector.scalar_tensor_tensor(
                    out=t1[:], in0=t1[:], scalar=cvec["lnx_w"][:, cc, :],
                    in1=p_rb[:], op0=OP.mult, op1=OP.mult)
                nc.vector.scalar_tensor_tensor(
                    out=ygT[:, cc, :], in0=t1[:],
                    scalar=cvec["lnx_b"][:, cc, :],
                    in1=gsT[:, cc, 1:NT], op0=OP.add, op1=OP.mult)
    if "d_yg" in dbg:
        tdy = pc.tile([P, NCC, NC_], F32, tag="tdy")
        nc.vector.tensor_copy(out=tdy[:], in_=ygT[:])
        nc.sync.dma_start(r3(dbg["d_yg"]), tdy[:])

    pool_gso.release()
    pool_d1 = tc.alloc_tile_pool(name="pool_d1", bufs=1, side="right")
    xre = pc.tile([P, NCC, NC_], F32, name="xre")
    nc.sync.dma_start(xre[:], r3(io["xT"])[:, :, 1:NT])
    x2 = pool_d1.tile([P, NCC, NC_], F32, name="x2")
    for oc in range(NCC):
        wtile = wpool.tile([P, NCC, P], BF16, tag="wtile")
        nc.sync.dma_start(
            wtile[:], io["wo"].ap()[oc].rearrange("(ic p) f -> p ic f", p=P))
        pm = pspool.tile([P, 512], F32, tag="proj", name="projp")[:, :NC_]
        for ic in range(NCC):
            nc.tensor.matmul(pm[:], wtile[:, ic, :], ygT[:, ic, :],
                             start=(ic == 0), stop=(ic == NCC - 1))
        nc.vector.tensor_tensor(out=x2[:, oc, :], in0=pm[:], in1=xre[:, oc, :],
                                op=OP.add)
    if "d_x2" in dbg:
        nc.sync.dma_start(r3(dbg["d_x2"]), x2[:])

    # =================================================================
    # Phase D: LN2, channel-mix shift, FFN
    # =================================================================
    pc.release()
    pool_ffw = tc.alloc_tile_pool(name="pool_ffw", bufs=2)
    pool_d2 = tc.alloc_tile_pool(name="pool_d2", bufs=1, side="right")
    xn2 = pool_d2.tile([P, NCC, NC_], F32, name="xn2")
    ln_layer(pool_d2, x2, NC_, cvec["ln2_w"], cvec["ln2_b"], xn2)
    ncm_t = pp.tile([P, NCC, 1], F32, name="ncm_t")
    nc.vector.tensor_copy(out=ncm_t[:], in_=xn2[:, :, NC_ - 1:NC_])
    nc.sync.dma_start(r3(io["ncm"]), ncm_t[:])
    nc.sync.dma_start(r3(io["nshift"]), sav_shift[:])
    if "d_xn2" in dbg:
        nc.sync.dma_start(r3(dbg["d_xn2"]), xn2[:])

    d2 = pool_d2.tile([P, NCC, NO], F32, tag="ln_sq", name="d2")
    nc.vector.tensor_tensor(out=d2[:], in0=xn2[:, :, 0:NO],
                            in1=xn2[:, :, 1:NO + 1], op=OP.subtract)
    u1b = pool_d2.tile([P, NCC, 1], F32, name="u1b")
    nc.vector.tensor_tensor(out=u1b[:], in0=c_cms[:], in1=xn2[:, :, 1:2],
                            op=OP.subtract)
    nc.vector.tensor_tensor(out=u1b[:], in0=u1b[:], in1=d2[:, :, 0:1],
                            op=OP.subtract)
    u2b = pool_d2.tile([P, NCC, 1], F32, name="u2b")
    for cc in range(NCC):
        nc.vector.scalar_tensor_tensor(
            out=u2b[:, cc, :], in0=u1b[:, cc, :], scalar=c_b0[:],
            in1=d2[:, cc, 0:1], op0=OP.mult, op1=OP.add)
    nc.vector.tensor_copy(out=d2[:, :, 0:1], in_=u2b[:])

    xk2 = pool_d1.tile([P, NCC, NO], BF16, name="xk2")
    xr2 = pool_d1.tile([P, NCC, NO], BF16, name="xr2")
    for cc in range(NCC):
        nc.vector.scalar_tensor_tensor(
            out=xk2[:, cc, :], in0=d2[:, cc, :],
            scalar=cvec["fmaa_k"][:, cc, :],
            in1=xn2[:, cc, 1:NO + 1], op0=OP.mult, op1=OP.add)
        nc.vector.scalar_tensor_tensor(
            out=xr2[:, cc, :], in0=d2[:, cc, :],
            scalar=cvec["fmaa_r"][:, cc, :],
            in1=xn2[:, cc, 1:NO + 1], op0=OP.mult, op1=OP.add)

    pool_d2.release()
    kk2 = pool_d1.tile([P, NFC, NO], BF16, name="kk2")
    for oc in range(NFC):
        wtile = wpool.tile([P, NCC, P], BF16, tag="wtile")
        nc.sync.dma_start(
            wtile[:], io["fwk"].ap()[oc].rearrange("(ic p) f -> p ic f", p=P))
        pm = pspool.tile([P, 512], F32, tag="proj", name="projp")[:, :NO]
        for ic in range(NCC):
            nc.tensor.matmul(pm[:], wtile[:, ic, :], xk2[:, ic, :],
                             start=(ic == 0), stop=(ic == NCC - 1))
        nc.scalar.activation(kk2[:, oc, :], pm[:], AF.Relu)
        nc.vector.tensor_tensor(out=kk2[:, oc, :], in0=kk2[:, oc, :],
                                in1=kk2[:, oc, :], op=OP.mult)

    kv2 = pool_d1.tile([P, NCC, NO], BF16, name="kv2")
    for oc in range(NCC):
        wtile = pool_ffw.tile([P, NFC, P], BF16, tag="wtile_ff")
        nc.sync.dma_start(
            wtile[:], io["fwv"].ap()[oc].rearrange("(ic p) f -> p ic f", p=P))
        pm = pspool.tile([P, 512], F32, tag="proj", name="projp")[:, :NO]
        for ic in range(NFC):
            nc.tensor.matmul(pm[:], wtile[:, ic, :], kk2[:, ic, :],
                             start=(ic == 0), stop=(ic == NFC - 1))
        nc.scalar.copy(kv2[:, oc, :], pm[:])
    sig = pool_d1.tile([P, NCC, NO], BF16, name="sig")
    for oc in range(NCC):
        wtile = wpool.tile([P, NCC, P], BF16, tag="wtile")
        nc.sync.dma_start(
            wtile[:], io["fwr"].ap()[oc].rearrange("(ic p) f -> p ic f", p=P))
        pm = pspool.tile([P, 512], F32, tag="proj", name="projp")[:, :NO]
        for ic in range(NCC):
            nc.tensor.matmul(pm[:], wtile[:, ic, :], xr2[:, ic, :],
                             start=(ic == 0), stop=(ic == NCC - 1))
        nc.scalar.activation(sig[:, oc, :], pm[:], AF.Sigmoid)
    nc.vector.tensor_tensor(out=kv2[:], in0=sig[:], in1=kv2[:], op=OP.mult)
    nc.vector.tensor_tensor(out=x2[:, :, 1:NO + 1], in0=kv2[:],
                            in1=x2[:, :, 1:NO + 1], op=OP.add)
    nc.sync.dma_start(r3(io["outx"]), x2[:, :, 1:NO + 1])
    pool_d1.release()
    pool_ffw.release()
    pspool.release()
    wpool.release()
    pp.release()


# ---------------------------------------------------------------------------
# Host side
# ---------------------------------------------------------------------------

def _tile_pk(w):
    """[C_in, C_out] -> [oc, ic*128, 128] bf16 contiguous lhsT tiles."""
    ci, co = w.shape
    t = w.reshape(ci // 128, 128, co // 128, 128)      # ic, p, oc, f
    t = np.ascontiguousarray(t.transpose(2, 0, 1, 3))  # oc, ic, p, f
    return bfc(t.reshape(co // 128, ci, 128))


def prep_inputs(inputs):
    inp = {k: np.asarray(v, dtype=np.float32) for k, v in inputs.items()}
    x = inp["x"]

    shared = {}
    shared["wr"] = _tile_pk(inp["Wr"].T)
    shared["wk"] = _tile_pk(inp["Wk"].T)
    shared["wv"] = _tile_pk(inp["Wv"].T)
    shared["wg"] = _tile_pk(inp["Wg"].T)
    shared["wo"] = _tile_pk(inp["Wo"].T)
    shared["fwr"] = _tile_pk(inp["fWr"].T)
    shared["fwk"] = _tile_pk(inp["fWk"].T)
    shared["fwv"] = _tile_pk(inp["fWv"].T)
    shared["maa_w1"] = bfc(inp["maa_w1"])
    maas = [inp[n].reshape(1, C) for n in
            ("maa_w", "maa_k", "maa_v", "maa_r", "maa_g")]
    shared["maa_w2a"] = bfc(np.stack(
        [np.concatenate([inp["maa_w2"][f], maas[f]], 0) for f in range(5)]))
    shared["td_w1"] = bfc(inp["td_w1"])
    shared["td_w2a"] = bfc(np.concatenate(
        [inp["td_w2"], inp["td"].reshape(1, C)], 0))
    col = lambda a: np.ascontiguousarray(a.reshape(C, 1), dtype=np.float32)
    shared["maa_x"] = col(inp["maa_x"])
    shared["u_col"] = col(inp["u"])
    for n in ("lnx_w", "lnx_b", "ln1_w", "ln1_b", "ln2_w", "ln2_b"):
        shared[n] = col(inp[n])
    shared["fmaa_k"] = col(inp["fmaa_k"])
    shared["fmaa_r"] = col(inp["fmaa_r"])
    shared["ones128"] = np.ones((128, 1), np.float32)
    shared["onesrow"] = np.ones((1, 128), np.float32)
    ind_m = np.zeros((NCC, 128, 32), np.float32)
    ind_mT = np.zeros((NCC, 32, 128), np.float32)
    for cc in range(NCC):
        for p in range(128):
            ind_m[cc, p, 2 * cc + p // 64] = 1.0
            ind_mT[cc, 2 * cc + p // 64, p] = 1.0
    shared["ind_m"] = ind_m
    shared["ind_mT"] = ind_mT
    um2 = np.zeros((128, L), np.float32)
    for p in range(128):
        um2[p, (p % L) + 1:] = 1.0
    shared["um2"] = um2
    i2_ = np.zeros((128, L), np.float32)
    for p in range(128):
        i2_[p, p % L] = 1.0
    shared["i2"] = i2_
    shared["id128"] = bfc(np.eye(128, dtype=np.float32))

    in_maps = []
    for core in range(NCORE):
        b, tq = core // TQ, core % TQ
        s = tq * TOK
        m = dict(shared)
        if tq == 0:
            xs = np.concatenate([x[b, 0:1], x[b, 0:1], x[b, 0:TOK]], 0)
        else:
            xs = x[b, s - 2:s + TOK]
        m["xT"] = np.ascontiguousarray(xs.T)
        m["s0"] = np.ascontiguousarray(inp["time_mix_state"][b].reshape(C, K))
        m["tms"] = np.ascontiguousarray(inp["time_mix_shift"][b].reshape(C, 1))
        m["cms"] = np.ascontiguousarray(
            inp["channel_mix_state"][b].reshape(C, 1))
        kwm = np.ones((128, NT), np.float32)
        kwm[:, 0] = 0.0
        if tq == 0:
            kwm[:, 1] = 0.0
        m["kwmask"] = kwm
        m["kwmaskn"] = -kwm
        m["blend0"] = np.full((128, 1), 1.0 if tq == 0 else 0.0, np.float32)
        pmv = np.zeros((128, 3), np.float32)
        pmv[:, :tq] = 1.0
        m["pmask"] = pmv
        in_maps.append(m)
    return in_maps


_NC_CACHE = {}


def get_nc(debug=False):
    if debug not in _NC_CACHE:
        _NC_CACHE[debug] = build(debug)
    return _NC_CACHE[debug]


def run(inputs, debug=False, **kw):
    nc = get_nc(debug)
    in_maps = prep_inputs(inputs)
    return run_bass_kernel_spmd(nc, in_maps, core_ids=list(range(NCORE)), **kw)


def assemble(res_list):
    x_out = np.zeros((B, T, C), np.float32)
    for core in range(NCORE):
        b, tq = core // TQ, core % TQ
        x_out[b, tq * TOK:(tq + 1) * TOK] = res_list[core]["outx"].T
    pick = [TQ - 1, NCORE - 1]
    new_shift = np.stack([res_list[c]["nshift"].reshape(1, C) for c in pick])
    new_cm = np.stack([res_list[c]["ncm"].reshape(1, C) for c in pick])
    new_wkv = np.stack([res_list[c]["nwkv"].reshape(H, K, K) for c in pick])
    return x_out, new_shift, new_cm, new_wkv


def kernel(**inputs):
    res = run(inputs)
    return assemble(res.results)


if __name__ == "__main__":
    build()
    print("build+compile OK")
